# revision 2
# baseline (speedup 1.0000x reference)
"""GCN AutoEncoder on 8 Trainium2 NeuronCores (Bass/Tile) — v2.

Key insight from profiling: dma_gather is SWDGE descriptor-generation bound
(~8ns/index on one Q7 pair), and gathers on different queue_nums are
generated by different Q7 pairs IN PARALLEL (measured 2.8x with 4 queues).

Design:
  - Nodes degree-sorted and dealt round-robin into 8 per-core partitions
    (identical compile-time tile structure across cores). x is sent to each
    core pre-permuted AND pre-transposed, so phase A is 4 stationary-xT
    matmuls per tile with no on-chip transposes.
  - Tables are fp16, PAIR-packed: table row pair j = nodes (2j, 2j+1),
    256B per pair. Pair-index space (25512 < 32768) fits int16 with NO
    lo/hi bucket split. Both layers share ONE index tensor and ONE edge
    structure (identical gathers).
  - Per dst, source slots are split by source-row parity (even pair-half /
    odd pair-half). Aggregation sums the wanted 64-lane half of each
    gathered pair directly on the PE via identity-stationary matmuls into
    PSUM (moving slices [128, cnt, 0:64] / [64:128]), then one
    tensor_reduce. The dis_dst normalization factor is applied in the
    per-tile epilogue (per-partition scale); dis_src is prescaled into the
    table entries. Self-loops are ordinary slots.
  - Gathers round-robin over 4 SWDGE queues so 4 Q7 pairs generate
    descriptors concurrently.
  - AllGather moves fp16 tables (half the bytes of f32).
"""

import os

import numpy as np

import concourse.bass as bass
import concourse.bacc as bacc
import concourse.mybir as mybir
import concourse.tile as tile
from concourse.instruction_name_ordered_set import InstructionNameOrderedSet
from concourse.bass_utils import run_bass_kernel_spmd

F32 = mybir.dt.float32
F16 = mybir.dt.float16
I16 = mybir.dt.int16

IN = 512
H = 64
L = 32
C = 8          # cores
NQ = 4         # SWDGE queues
ZROWS = 128    # zero rows per core shard (64 zero pairs, padding targets)
GCAP = 72      # max slots per gather instruction (<= ~9216 descriptors)

LAST_RESULTS = None


class Sizes:
    def __init__(self, n):
        self.N = n
        self.NPC = n // C
        assert self.NPC * C == n and self.NPC % 2 == 0
        self.SROWS = self.NPC + ZROWS
        self.TROWS = self.SROWS * C
        self.PAIRS = self.TROWS // 2
        assert self.PAIRS < 32768
        self.NT = (self.NPC + 127) // 128
        self.TSZ = [128] * (self.NT - 1) + [self.NPC - (self.NT - 1) * 128]


def _wrap_idx(arr_k128):
    """Slot-major [K,128] -> wrapped [128, K*8] int16 (idx i at [i%16, i//16],
    replicated across the 8 groups of 16 partitions)."""
    flat = arr_k128.reshape(-1)
    w16 = flat.reshape(-1, 16).T
    return np.tile(w16, (8, 1)).astype(np.int16)


def _preprocess(sz, edge_index):
    n = sz.N
    src = np.asarray(edge_index[0], dtype=np.int64)
    dst = np.asarray(edge_index[1], dtype=np.int64)
    deg = np.bincount(dst, minlength=n).astype(np.int64) + 1
    dis = (1.0 / np.sqrt(deg.astype(np.float64))).astype(np.float32)

    # CSR over dst including self-loops
    srcs_all = np.concatenate([src, np.arange(n, dtype=np.int64)])
    dsts_all = np.concatenate([dst, np.arange(n, dtype=np.int64)])
    order = np.argsort(dsts_all, kind="stable")
    srcs_sorted = srcs_all[order]
    indptr = np.zeros(n + 1, dtype=np.int64)
    np.cumsum(np.bincount(dsts_all, minlength=n), out=indptr[1:])

    # partition nodes into tiles + assign pair-half parities.
    oorder = np.argsort(srcs_all, kind="stable")
    out_dst = dsts_all[oorder]
    out_ptr = np.zeros(n + 1, dtype=np.int64)
    np.cumsum(np.bincount(srcs_all, minlength=n), out=out_ptr[1:])
    odeg_all = np.diff(out_ptr)
    deg1 = np.diff(indptr)

    def deal(keys):
        """Sort nodes by key desc, deal per tile across cores."""
        pord = np.argsort(-keys, kind="stable")
        pt = [[] for _ in range(C)]
        off = 0
        for t in range(sz.NT):
            g = pord[off: off + sz.TSZ[t] * C]
            off += sz.TSZ[t] * C
            for c in range(C):
                pt[c].append(g[c * sz.TSZ[t]: (c + 1) * sz.TSZ[t]])
        return pt

    def balance(perm_tiles, parity_init=None, bal_init=None):
        """Greedy + swap-refined parity assignment under per-tile budgets."""
        tile_of = np.empty(n, dtype=np.int64)
        for c in range(C):
            for t in range(sz.NT):
                tile_of[perm_tiles[c][t]] = c * sz.NT + t
        ntiles = C * sz.NT
        budget_e = np.array([(len(perm_tiles[c][t]) + 1) // 2
                             for c in range(C) for t in range(sz.NT)])
        budget_o = np.array([len(perm_tiles[c][t])
                             for c in range(C) for t in range(sz.NT)]) - budget_e
        used_e = np.zeros(ntiles, dtype=np.int64)
        used_o = np.zeros(ntiles, dtype=np.int64)
        if parity_init is not None:
            # warm start: keep parities, repair per-tile budget violations
            parity = parity_init.copy()
            bal = bal_init.copy()
            for tl in range(ntiles):
                c, t = divmod(tl, sz.NT)
                nodes = perm_tiles[c][t]
                ev = nodes[parity[nodes] == 0]
                od = nodes[parity[nodes] == 1]
                while len(ev) > budget_e[tl]:
                    g = np.array([(1 - bal[out_dst[out_ptr[v]: out_ptr[v+1]]]).sum() for v in ev])
                    v = ev[np.argmin(g)]
                    parity[v] = 1
                    bal[out_dst[out_ptr[v]: out_ptr[v+1]]] -= 2
                    ev = ev[ev != v]
                while len(od) > budget_o[tl]:
                    g = np.array([(1 + bal[out_dst[out_ptr[v]: out_ptr[v+1]]]).sum() for v in od])
                    v = od[np.argmin(g)]
                    parity[v] = 0
                    bal[out_dst[out_ptr[v]: out_ptr[v+1]]] += 2
                    od = od[od != v]
            names_skip_pass1 = True
        else:
            names_skip_pass1 = False
            bal = np.zeros(n, dtype=np.int32)
            parity = np.zeros(n, dtype=np.int8)
        for node in (() if names_skip_pass1 else np.argsort(-odeg_all, kind="stable")):
            tl = tile_of[node]
            ds = out_dst[out_ptr[node]: out_ptr[node + 1]]
            b = bal[ds]
            score_e = int((b >= 0).sum()) - int((b < 0).sum())
            score_o = int((b <= 0).sum()) - int((b > 0).sum())
            pick_e = (score_e <= score_o)
            if pick_e and used_e[tl] >= budget_e[tl]:
                pick_e = False
            elif not pick_e and used_o[tl] >= budget_o[tl]:
                pick_e = True
            if pick_e:
                parity[node] = 0
                used_e[tl] += 1
                bal[ds] += 1
            else:
                parity[node] = 1
                used_o[tl] += 1
                bal[ds] -= 1
        for _ in range(3):
            nswap = 0
            for tl in range(ntiles):
                c, t = divmod(tl, sz.NT)
                nodes = perm_tiles[c][t]
                ev = nodes[parity[nodes] == 0]
                od = nodes[parity[nodes] == 1]
                ge = np.array([(1 - bal[out_dst[out_ptr[v]: out_ptr[v + 1]]]).sum()
                               for v in ev])
                go = np.array([(1 + bal[out_dst[out_ptr[v]: out_ptr[v + 1]]]).sum()
                               for v in od])
                eo = np.argsort(ge)
                oo = np.argsort(go)
                for i in range(min(len(ev), len(od))):
                    a_, b_ = ev[eo[i]], od[oo[i]]
                    da = out_dst[out_ptr[a_]: out_ptr[a_ + 1]]
                    db = out_dst[out_ptr[b_]: out_ptr[b_ + 1]]
                    gain = (1 - bal[da]).sum() + (1 + bal[db]).sum()
                    if gain >= 0:
                        break
                    parity[a_], parity[b_] = 1, 0
                    bal[da] -= 2
                    bal[db] += 2
                    nswap += 1
            if nswap == 0:
                break
        return parity, bal

    perm_tiles = deal(deg)
    parity, bal = balance(perm_tiles)
    # re-deal by the binding dimension max(n_even, n_odd), re-balance
    ne_d = (deg1 + bal) // 2
    no_d = (deg1 - bal) // 2
    perm_tiles = deal(np.maximum(ne_d, no_d) * 64 + np.minimum(ne_d, no_d))
    parity, bal = balance(perm_tiles, parity_init=parity, bal_init=bal)

    # order each tile: evens at positions 0,2,4..., odds at 1,3,5...
    for c in range(C):
        for t in range(sz.NT):
            nodes = perm_tiles[c][t]
            ev = nodes[parity[nodes] == 0]
            od = nodes[parity[nodes] == 1]
            arr = np.empty(len(nodes), dtype=np.int64)
            arr[0: 2 * len(ev): 2] = ev
            arr[1: 2 * len(od) + 1: 2] = od
            perm_tiles[c][t] = arr
    perm_nodes = [np.concatenate(p) for p in perm_tiles]

    row = np.empty(n, dtype=np.int64)
    for c in range(C):
        row[perm_nodes[c]] = c * sz.SROWS + np.arange(sz.NPC)

    rows_of_srcs = row[srcs_sorted]          # source rows per CSR entry
    pair_of_srcs = rows_of_srcs >> 1
    par_of_srcs = (rows_of_srcs & 1).astype(np.int64)

    # even/odd source counts per node
    seg_id = np.repeat(np.arange(n), deg1)
    n_odd = np.bincount(seg_id, weights=par_of_srcs, minlength=n).astype(np.int64)
    n_even = deg1 - n_odd
    # sort each node's CSR segment: evens first, then odds
    order2 = np.lexsort((par_of_srcs, seg_id))
    pairs_s = pair_of_srcs[order2]

    # per-tile K (max over the 8 cores' tile-t nodes; program is SPMD)
    Ke, Ko = [], []
    for t in range(sz.NT):
        gnodes = np.concatenate(
            [perm_nodes[c][t * 128: t * 128 + sz.TSZ[t]] for c in range(C)])
        Ke.append(max(1, int(n_even[gnodes].max())))
        Ko.append(max(1, int(n_odd[gnodes].max())))

    # group tiles into gather instructions of <= GCAP slots
    groups = []
    cur, s = [], 0
    for t in range(sz.NT):
        kt = Ke[t] + Ko[t]
        if cur and s + kt > GCAP:
            groups.append(cur)
            cur, s = [], 0
        cur.append(t)
        s += kt
    if cur:
        groups.append(cur)

    # zero-pair block of core 0 (exists in every core's table copy)
    zpair0 = (sz.NPC) // 2
    zpairs = ZROWS // 2

    def tile_block(c, t):
        ke, ko = Ke[t], Ko[t]
        nodes = perm_nodes[c][t * 128: t * 128 + sz.TSZ[t]]
        spread = (np.arange(128)[:, None] * 7 + np.arange(ke + ko)[None, :]) % zpairs
        arr = (zpair0 + spread.astype(np.int64)).T.copy()   # [K,128] padding
        for j, n_ in enumerate(nodes):
            a = indptr[n_]
            ne = deg1[n_] - n_odd[n_]
            arr[0:ne, j] = pairs_s[a: a + ne]
            arr[ke: ke + n_odd[n_], j] = pairs_s[a + ne: a + deg1[n_]]
        return _wrap_idx(arr)

    idx_tensors = []
    for c in range(C):
        blocks = []
        for grp in groups:
            for t in grp:
                blocks.append(tile_block(c, t))
        idx_tensors.append(np.concatenate(blocks, axis=1))

    disp = np.zeros((C, 128, sz.NT), dtype=np.float32)
    for c in range(C):
        for t in range(sz.NT):
            disp[c, : sz.TSZ[t], t] = dis[perm_nodes[c][t * 128: t * 128 + sz.TSZ[t]]]

    return dict(perm_nodes=perm_nodes, Ke=Ke, Ko=Ko, groups=groups,
                idx=idx_tensors, disp=disp)


# -------------------------------------------------------------- device side

def build_program(nc, sz, meta):
    NPC, NT, TSZ = sz.NPC, sz.NT, sz.TSZ
    Ke, Ko, groups = meta["Ke"], meta["Ko"], meta["groups"]
    CW = sum((Ke[t] + Ko[t]) * 8 for t in range(NT))

    xT = nc.dram_tensor("xT", [IN, NPC], F32, kind="ExternalInput")
    idx = nc.dram_tensor("idx", [128, CW], I16, kind="ExternalInput")
    disp_d = nc.dram_tensor("disp", [128, NT], F32, kind="ExternalInput")
    dispw_d = nc.dram_tensor("dispw", [128, NT * H], F32, kind="ExternalInput")
    w1 = nc.dram_tensor("w1", [IN, H], F32, kind="ExternalInput")
    b1bc_d = nc.dram_tensor("b1bc", [128, H], F32, kind="ExternalInput")
    w2 = nc.dram_tensor("w2", [H, L], F32, kind="ExternalInput")
    b2_d = nc.dram_tensor("b2", [L, 1], F32, kind="ExternalInput")
    wd1 = nc.dram_tensor("wd1", [L, H], F32, kind="ExternalInput")
    bd1_d = nc.dram_tensor("bd1", [H, 1], F32, kind="ExternalInput")
    wd2 = nc.dram_tensor("wd2", [H, IN], F32, kind="ExternalInput")
    bd2_d = nc.dram_tensor("bd2", [128, 4], F32, kind="ExternalInput")
    ident_d = nc.dram_tensor("ident", [128, 128], F32, kind="ExternalInput")
    out = nc.dram_tensor("out", [NPC, IN], F32, kind="ExternalOutput")

    ACT = mybir.ActivationFunctionType
    ADD = mybir.AluOpType.add
    MULT = mybir.AluOpType.mult
    rg = [list(range(C))]

    table1 = nc.dram_tensor("table1", [sz.TROWS, H], F16, kind="Internal",
                            addr_space="Shared")
    table2 = nc.dram_tensor("table2", [sz.TROWS, H], F16, kind="Internal",
                            addr_space="Shared")

    with tile.TileContext(nc) as tc:
        with (
            tc.tile_pool(name="const", bufs=1) as cpool,
            tc.tile_pool(name="dram", bufs=1, space="DRAM") as dpool,
        ):
            cc1 = dpool.tile([sz.SROWS, H], F16)
            cc2 = dpool.tile([sz.SROWS, H], F16)
            ident = cpool.tile([128, 128], F32)
            nc.sync.dma_start(ident[:], ident_d.ap())
            ident16 = cpool.tile([128, 128], F16)
            nc.vector.tensor_copy(ident16[:], ident[:])
            b1bc = cpool.tile([128, H], F32)
            nc.sync.dma_start(b1bc[:], b1bc_d.ap())
            w1_sb = cpool.tile([128, 4, H], F32)
            nc.sync.dma_start(w1_sb[:], w1.ap().rearrange("(b p) f -> p b f", p=128))
            w2_sb = cpool.tile([H, L], F32)
            nc.sync.dma_start(w2_sb[:], w2.ap())
            b2_sb = cpool.tile([L, 1], F32)
            nc.sync.dma_start(b2_sb[:], b2_d.ap())
            wd1_sb = cpool.tile([L, H], F32)
            nc.sync.dma_start(wd1_sb[:], wd1.ap())
            bd1_sb = cpool.tile([H, 1], F32)
            nc.sync.dma_start(bd1_sb[:], bd1_d.ap())
            wd2_sb = cpool.tile([H, IN], F32)
            nc.sync.dma_start(wd2_sb[:], wd2.ap())
            bd2_sb = cpool.tile([128, 4], F32)
            nc.sync.dma_start(bd2_sb[:], bd2_d.ap())
            disp = cpool.tile([128, NT], F32)
            nc.sync.dma_start(disp[:], disp_d.ap())
            dispw = cpool.tile([128, NT, H], F32)
            nc.scalar.dma_start(dispw[:], dispw_d.ap().rearrange(
                "p (t f) -> p t f", f=H))
            idx_sb = cpool.tile([128, CW], I16)
            nc.scalar.dma_start(idx_sb[:], idx.ap())

            # zero rows of cc1/cc2 (padding gather targets)
            zb = ZROWS // 128
            zrow = cpool.tile([128, zb, H], F16)
            nc.vector.memset(zrow[:], 0.0)
            nc.sync.dma_start(
                cc1[NPC: sz.SROWS, :].rearrange("(p b) f -> p b f", p=128),
                zrow[:])
            nc.sync.dma_start(
                cc2[NPC: sz.SROWS, :].rearrange("(p b) f -> p b f", p=128),
                zrow[:])

            # ---------------- phase A: cc1 = fp16( dis * (x @ W1) )
            # x arrives pre-transposed; two big HWDGE loads (half the shard
            # each). Tiles are processed in batches of 8 per PSUM bank; one
            # wide DVE multiply applies the per-(tile,dst) dis scale and
            # casts to fp16, and one DMA stores the whole batch.
            NHALF = (NT + 1) // 2
            HROWS = NHALF * 128
            BT = 8
            with (
                tc.tile_pool(name="pa_x", bufs=2) as pax,
                tc.tile_pool(name="pa_sb", bufs=3) as pa,
                tc.tile_pool(name="pa_ps", bufs=3, space="PSUM") as pap,
            ):
                for half in range(2):
                    r0 = half * HROWS
                    r1 = min(NPC, (half + 1) * HROWS)
                    xh = pax.tile([128, 4, HROWS], F32, tag="xh")
                    nc.sync.dma_start(
                        xh[:, :, : r1 - r0],
                        xT.ap()[:, r0: r1].rearrange("(b k) d -> k b d", k=128))
                    tlo = half * NHALF
                    thi = min(NT, (half + 1) * NHALF)
                    for tb in range(tlo, thi, BT):
                        nb = min(BT, thi - tb)
                        ps = pap.tile([128, BT, H], F32, tag="ps")
                        for i in range(nb):
                            t = tb + i
                            pn = TSZ[t]
                            o = t * 128 - r0
                            for b in range(4):
                                nc.tensor.matmul(
                                    ps[:pn, i, :], xh[:, b, o: o + pn],
                                    w1_sb[:, b, :],
                                    start=(b == 0), stop=(b == 3),
                                )
                        ccb = pa.tile([128, BT, H], F16, tag="ccb")
                        nc.vector.tensor_tensor(
                            ccb[:, :nb, :], ps[:, :nb, :],
                            dispw[:, tb: tb + nb, :], op=MULT)
                        nfull = nb if tb + nb < NT else nb - 1
                        if nfull > 0:
                            nc.scalar.dma_start(
                                cc1[tb * 128: (tb + nfull) * 128, :].rearrange(
                                    "(t p) f -> p t f", p=128),
                                ccb[:, :nfull, :])
                        if tb + nb == NT:
                            pn = TSZ[NT - 1]
                            nc.scalar.dma_start(
                                cc1[(NT - 1) * 128: (NT - 1) * 128 + pn, :],
                                ccb[:pn, nb - 1, :])

            # ---------------- AllGather 1
            nc.gpsimd.collective_compute(
                "AllGather", mybir.AluOpType.bypass, replica_groups=rg,
                ins=[cc1.opt()], outs=[table1.ap()],
            )

            # ---------------- aggregation helper
            SMAX = max(sum(Ke[t] + Ko[t] for t in grp) for grp in groups)
            qctr = {"gi": 0}

            def agg_layer(table, epi, pg, pp, pu):
                in_ap = table.ap().rearrange("(a b) f -> a (b f)", b=2)
                col = 0
                for grp in groups:
                    S = sum(Ke[t] + Ko[t] for t in grp)
                    g = pg.tile([128, SMAX, 128], F16, tag="g")
                    nc.gpsimd.dma_gather(
                        out_ap=g[:, :S, :],
                        in_ap=in_ap,
                        idxs_ap=idx_sb[:, col: col + S * 8],
                        num_idxs=S * 128,
                        num_idxs_reg=S * 128,
                        elem_size=128,
                        single_packet=False,
                        queue_num=qctr["gi"] % NQ,
                    )
                    qctr["gi"] += 1
                    o = 0
                    for t in grp:
                        psum8 = pp.tile([128, 512], F32, tag="psum8")
                        first = True
                        for off, K, lo in ((o, Ke[t], 0), (o + Ke[t], Ko[t], 64)):
                            nmm = (K + 7) // 8
                            for j in range(nmm):
                                cnt = min(8, K - j * 8)
                                nc.tensor.matmul(
                                    psum8[:, : cnt * H],
                                    ident16[:],
                                    g[:, off + j * 8: off + j * 8 + cnt,
                                      lo: lo + H],
                                    start=first,
                                    stop=(lo == 64 and j == nmm - 1),
                                )
                                first = False
                        o += Ke[t] + Ko[t]
                        u = pu.tile([128, H], F32, tag="u")
                        nc.vector.tensor_reduce(
                            u[:],
                            psum8[:].rearrange("p (k f) -> p f k", k=8),
                            axis=mybir.AxisListType.X,
                            op=ADD,
                        )
                        epi(t, u)
                    col += S * 8

            # ---------------- layer 1 aggregation -> cc2
            with (
                tc.tile_pool(name="pc_g", bufs=4) as pcg,
                tc.tile_pool(name="pc_u", bufs=3) as pcu,
                tc.tile_pool(name="pc_ps", bufs=3, space="PSUM") as pcp,
            ):
                def epi_c(t, u):
                    # hs2 = dis * relu(dis*u + b1); relu commutes with the
                    # nonneg dis scale, so: t1 = dis*u (ACT), hpre = t1+b1
                    # (DVE add), hs2 = relu(dis*hpre... ) -- careful: the
                    # final dis must multiply relu(hpre), and ACT computes
                    # func(scale*in + bias), so Relu with scale=dis gives
                    # relu(dis*hpre) = dis*relu(hpre) since dis >= 0.
                    pn = TSZ[t]
                    t1 = pcu.tile([128, H], F32, tag="t1")
                    nc.scalar.activation(t1[:], u[:], ACT.Copy,
                                         scale=disp[:, t: t + 1])
                    hpre = pcu.tile([128, H], F32, tag="hpre")
                    nc.vector.tensor_tensor(hpre[:], t1[:], b1bc[:], op=ADD)
                    hs2 = pcu.tile([128, H], F16, tag="hs2")
                    nc.scalar.activation(hs2[:], hpre[:], ACT.Relu,
                                         scale=disp[:, t: t + 1])
                    nc.sync.dma_start(cc2[t * 128: t * 128 + pn, :],
                                      hs2[:pn, :])

                agg_layer(table1, epi_c, pcg, pcp, pcu)

            # ---------------- AllGather 2
            nc.gpsimd.collective_compute(
                "AllGather", mybir.AluOpType.bypass, replica_groups=rg,
                ins=[cc2.opt()], outs=[table2.ap()],
            )

            # ---------------- layer 2 aggregation + decoder -> out
            with (
                tc.tile_pool(name="pe_g", bufs=4) as peg,
                tc.tile_pool(name="pe_u", bufs=2) as peu,
                tc.tile_pool(name="pe_o", bufs=2) as peo,
                tc.tile_pool(name="pe_ps8", bufs=2, space="PSUM") as pep8,
                tc.tile_pool(name="pe_pss", bufs=3, space="PSUM") as peps,
                tc.tile_pool(name="pe_pso", bufs=3, space="PSUM") as pepo,
            ):
                def epi_e(t, u2):
                    pn = TSZ[t]
                    u2s = peu.tile([128, H], F32, tag="u2s")
                    nc.scalar.activation(u2s[:], u2[:], ACT.Copy,
                                         scale=disp[:, t: t + 1])
                    ps_uT = peps.tile([H, 128], F32, tag="ps_small")
                    nc.tensor.transpose(ps_uT[:], u2s[:], ident[:])
                    uT = peu.tile([H, 128], F32, tag="uT")
                    nc.vector.tensor_copy(uT[:], ps_uT[:])
                    ps_z = peps.tile([H, 128], F32, tag="ps_small")
                    nc.tensor.matmul(ps_z[:L, :], w2_sb[:], uT[:],
                                     start=True, stop=True)
                    zT = peu.tile([L, 128], F32, tag="zT")
                    nc.scalar.activation(zT[:], ps_z[:L, :], ACT.Relu,
                                         bias=b2_sb[:])
                    ps_d = peps.tile([H, 128], F32, tag="ps_small")
                    nc.tensor.matmul(ps_d[:], wd1_sb[:], zT[:],
                                     start=True, stop=True)
                    dT = peu.tile([H, 128], F32, tag="dT")
                    nc.scalar.activation(dT[:], ps_d[:], ACT.Relu,
                                         bias=bd1_sb[:])
                    osb = peo.tile([128, IN], F32, tag="osb")
                    for gblk in range(4):
                        ps_o = pepo.tile([128, 128], F32, tag="ps_oo")
                        nc.tensor.matmul(
                            ps_o[:],
                            wd2_sb[:, gblk * 128:(gblk + 1) * 128], dT[:],
                            start=True, stop=True,
                        )
                        oT = peu.tile([128, 128], F32, tag="oT")
                        nc.scalar.activation(
                            oT[:], ps_o[:], ACT.Sigmoid,
                            bias=bd2_sb[:, gblk: gblk + 1],
                        )
                        ps_on = pepo.tile([128, 128], F32, tag="ps_oo")
                        nc.tensor.transpose(ps_on[:], oT[:], ident[:])
                        nc.vector.tensor_copy(
                            osb[:, gblk * 128:(gblk + 1) * 128], ps_on[:]
                        )
                    nc.sync.dma_start(out.ap()[t * 128: t * 128 + pn, :],
                                      osb[:pn, :])

                agg_layer(table2, epi_e, peg, pep8, peu)
    return nc


def make_in_maps(sz, meta, x, W1, b1, W2, b2, Wd1, bd1, Wd2, bd2):
    b1bc = np.tile(b1[None, :], (128, 1)).astype(np.float32)
    bd2t = bd2.reshape(4, 128).T.copy().astype(np.float32)
    ident = np.eye(128, dtype=np.float32)
    in_maps = []
    for c in range(C):
        xp = np.ascontiguousarray(x[meta["perm_nodes"][c]].T)
        in_maps.append({
            "xT": xp,
            "idx": np.ascontiguousarray(meta["idx"][c]),
            "disp": np.ascontiguousarray(meta["disp"][c]),
            "dispw": np.ascontiguousarray(
                np.repeat(meta["disp"][c], H, axis=1).astype(np.float32)),
            "w1": W1, "b1bc": b1bc, "w2": W2,
            "b2": b2.reshape(L, 1).astype(np.float32),
            "wd1": Wd1, "bd1": bd1.reshape(H, 1).astype(np.float32),
            "wd2": Wd2, "bd2": bd2t, "ident": ident,
        })
    return in_maps


# ------------------------------------------------------------------- driver

def kernel(**inputs):
    x = np.ascontiguousarray(np.asarray(inputs["x"], dtype=np.float32))
    edge_index = np.asarray(inputs["edge_index"])
    args = [np.asarray(inputs[k], dtype=np.float32)
            for k in ["W1", "b1", "W2", "b2", "Wd1", "bd1", "Wd2", "bd2"]]

    sz = Sizes(x.shape[0])
    meta = _preprocess(sz, edge_index)

    nc = bacc.Bacc("TRN2", target_bir_lowering=False, debug=False,
                   num_devices=C, num_swdge_queues=NQ)
    build_program(nc, sz, meta)
    nc.compile()

    in_maps = make_in_maps(sz, meta, x, *args)

    trace = bool(int(os.environ.get("GCN_TRACE", "0")))
    res = run_bass_kernel_spmd(nc, in_maps, core_ids=list(range(C)), trace=trace)
    global LAST_RESULTS
    LAST_RESULTS = res

    out_full = np.empty((sz.N, IN), dtype=np.float32)
    for c in range(C):
        out_full[meta["perm_nodes"][c]] = res.results[c]["out"]
    return out_full


# revision 3
# speedup vs baseline: 1.1320x; 1.1320x over previous
"""GCN AutoEncoder on 8 Trainium2 NeuronCores (Bass/Tile) — v2.

Key insight from profiling: dma_gather is SWDGE descriptor-generation bound
(~8ns/index on one Q7 pair), and gathers on different queue_nums are
generated by different Q7 pairs IN PARALLEL (measured 2.8x with 4 queues).

Design:
  - Nodes degree-sorted and dealt round-robin into 8 per-core partitions
    (identical compile-time tile structure across cores). x is sent to each
    core pre-permuted AND pre-transposed, so phase A is 4 stationary-xT
    matmuls per tile with no on-chip transposes.
  - Tables are fp16, PAIR-packed: table row pair j = nodes (2j, 2j+1),
    256B per pair. Pair-index space (25512 < 32768) fits int16 with NO
    lo/hi bucket split. Both layers share ONE index tensor and ONE edge
    structure (identical gathers).
  - Per dst, source slots are split by source-row parity (even pair-half /
    odd pair-half). Aggregation sums the wanted 64-lane half of each
    gathered pair directly on the PE via identity-stationary matmuls into
    PSUM (moving slices [128, cnt, 0:64] / [64:128]), then one
    tensor_reduce. The dis_dst normalization factor is applied in the
    per-tile epilogue (per-partition scale); dis_src is prescaled into the
    table entries. Self-loops are ordinary slots.
  - Gathers round-robin over 4 SWDGE queues so 4 Q7 pairs generate
    descriptors concurrently.
  - AllGather moves fp16 tables (half the bytes of f32).
"""

import os

import numpy as np

import concourse.bass as bass
import concourse.bacc as bacc
import concourse.mybir as mybir
import concourse.tile as tile
from concourse.instruction_name_ordered_set import InstructionNameOrderedSet
from concourse.bass_utils import run_bass_kernel_spmd

F32 = mybir.dt.float32
F16 = mybir.dt.float16
I16 = mybir.dt.int16

IN = 512
H = 64
L = 32
C = 8          # cores
NQ = 4         # SWDGE queues
ZROWS = 128    # zero rows per core shard (64 zero pairs, padding targets)
GCAP = 72      # max slots per gather instruction (<= ~9216 descriptors)

LAST_RESULTS = None


class Sizes:
    def __init__(self, n):
        self.N = n
        self.NPC = n // C
        assert self.NPC * C == n and self.NPC % 2 == 0
        self.SROWS = self.NPC + ZROWS
        self.TROWS = self.SROWS * C
        self.PAIRS = self.TROWS // 2
        assert self.PAIRS < 32768
        self.NT = (self.NPC + 127) // 128
        self.TSZ = [128] * (self.NT - 1) + [self.NPC - (self.NT - 1) * 128]


def _wrap_idx(arr_k128):
    """Slot-major [K,128] -> wrapped [128, K*8] int16 (idx i at [i%16, i//16],
    replicated across the 8 groups of 16 partitions)."""
    flat = arr_k128.reshape(-1)
    w16 = flat.reshape(-1, 16).T
    return np.tile(w16, (8, 1)).astype(np.int16)


def _preprocess(sz, edge_index):
    n = sz.N
    src = np.asarray(edge_index[0], dtype=np.int64)
    dst = np.asarray(edge_index[1], dtype=np.int64)
    deg = np.bincount(dst, minlength=n).astype(np.int64) + 1
    dis = (1.0 / np.sqrt(deg.astype(np.float64))).astype(np.float32)

    # CSR over dst including self-loops
    srcs_all = np.concatenate([src, np.arange(n, dtype=np.int64)])
    dsts_all = np.concatenate([dst, np.arange(n, dtype=np.int64)])
    order = np.argsort(dsts_all, kind="stable")
    srcs_sorted = srcs_all[order]
    indptr = np.zeros(n + 1, dtype=np.int64)
    np.cumsum(np.bincount(dsts_all, minlength=n), out=indptr[1:])

    # partition nodes into tiles + assign pair-half parities.
    oorder = np.argsort(srcs_all, kind="stable")
    out_dst = dsts_all[oorder]
    out_ptr = np.zeros(n + 1, dtype=np.int64)
    np.cumsum(np.bincount(srcs_all, minlength=n), out=out_ptr[1:])
    odeg_all = np.diff(out_ptr)
    deg1 = np.diff(indptr)

    def deal(keys):
        """Sort nodes by key desc, deal per tile across cores."""
        pord = np.argsort(-keys, kind="stable")
        pt = [[] for _ in range(C)]
        off = 0
        for t in range(sz.NT):
            g = pord[off: off + sz.TSZ[t] * C]
            off += sz.TSZ[t] * C
            for c in range(C):
                pt[c].append(g[c * sz.TSZ[t]: (c + 1) * sz.TSZ[t]])
        return pt

    def balance(perm_tiles, parity_init=None, bal_init=None):
        """Greedy + swap-refined parity assignment under per-tile budgets."""
        tile_of = np.empty(n, dtype=np.int64)
        for c in range(C):
            for t in range(sz.NT):
                tile_of[perm_tiles[c][t]] = c * sz.NT + t
        ntiles = C * sz.NT
        budget_e = np.array([(len(perm_tiles[c][t]) + 1) // 2
                             for c in range(C) for t in range(sz.NT)])
        budget_o = np.array([len(perm_tiles[c][t])
                             for c in range(C) for t in range(sz.NT)]) - budget_e
        used_e = np.zeros(ntiles, dtype=np.int64)
        used_o = np.zeros(ntiles, dtype=np.int64)
        if parity_init is not None:
            # warm start: keep parities, repair per-tile budget violations
            parity = parity_init.copy()
            bal = bal_init.copy()
            for tl in range(ntiles):
                c, t = divmod(tl, sz.NT)
                nodes = perm_tiles[c][t]
                ev = nodes[parity[nodes] == 0]
                od = nodes[parity[nodes] == 1]
                while len(ev) > budget_e[tl]:
                    g = np.array([(1 - bal[out_dst[out_ptr[v]: out_ptr[v+1]]]).sum() for v in ev])
                    v = ev[np.argmin(g)]
                    parity[v] = 1
                    bal[out_dst[out_ptr[v]: out_ptr[v+1]]] -= 2
                    ev = ev[ev != v]
                while len(od) > budget_o[tl]:
                    g = np.array([(1 + bal[out_dst[out_ptr[v]: out_ptr[v+1]]]).sum() for v in od])
                    v = od[np.argmin(g)]
                    parity[v] = 0
                    bal[out_dst[out_ptr[v]: out_ptr[v+1]]] += 2
                    od = od[od != v]
            names_skip_pass1 = True
        else:
            names_skip_pass1 = False
            bal = np.zeros(n, dtype=np.int32)
            parity = np.zeros(n, dtype=np.int8)
        for node in (() if names_skip_pass1 else np.argsort(-odeg_all, kind="stable")):
            tl = tile_of[node]
            ds = out_dst[out_ptr[node]: out_ptr[node + 1]]
            b = bal[ds]
            score_e = int((b >= 0).sum()) - int((b < 0).sum())
            score_o = int((b <= 0).sum()) - int((b > 0).sum())
            pick_e = (score_e <= score_o)
            if pick_e and used_e[tl] >= budget_e[tl]:
                pick_e = False
            elif not pick_e and used_o[tl] >= budget_o[tl]:
                pick_e = True
            if pick_e:
                parity[node] = 0
                used_e[tl] += 1
                bal[ds] += 1
            else:
                parity[node] = 1
                used_o[tl] += 1
                bal[ds] -= 1
        for _ in range(3):
            nswap = 0
            for tl in range(ntiles):
                c, t = divmod(tl, sz.NT)
                nodes = perm_tiles[c][t]
                ev = nodes[parity[nodes] == 0]
                od = nodes[parity[nodes] == 1]
                ge = np.array([(1 - bal[out_dst[out_ptr[v]: out_ptr[v + 1]]]).sum()
                               for v in ev])
                go = np.array([(1 + bal[out_dst[out_ptr[v]: out_ptr[v + 1]]]).sum()
                               for v in od])
                eo = np.argsort(ge)
                oo = np.argsort(go)
                for i in range(min(len(ev), len(od))):
                    a_, b_ = ev[eo[i]], od[oo[i]]
                    da = out_dst[out_ptr[a_]: out_ptr[a_ + 1]]
                    db = out_dst[out_ptr[b_]: out_ptr[b_ + 1]]
                    gain = (1 - bal[da]).sum() + (1 + bal[db]).sum()
                    if gain >= 0:
                        break
                    parity[a_], parity[b_] = 1, 0
                    bal[da] -= 2
                    bal[db] += 2
                    nswap += 1
            if nswap == 0:
                break
        return parity, bal

    perm_tiles = deal(deg)
    parity, bal = balance(perm_tiles)
    # re-deal by the binding dimension max(n_even, n_odd), re-balance
    ne_d = (deg1 + bal) // 2
    no_d = (deg1 - bal) // 2
    perm_tiles = deal(np.maximum(ne_d, no_d) * 64 + np.minimum(ne_d, no_d))
    parity, bal = balance(perm_tiles, parity_init=parity, bal_init=bal)

    # order each tile: evens at positions 0,2,4..., odds at 1,3,5...
    for c in range(C):
        for t in range(sz.NT):
            nodes = perm_tiles[c][t]
            ev = nodes[parity[nodes] == 0]
            od = nodes[parity[nodes] == 1]
            arr = np.empty(len(nodes), dtype=np.int64)
            arr[0: 2 * len(ev): 2] = ev
            arr[1: 2 * len(od) + 1: 2] = od
            perm_tiles[c][t] = arr
    perm_nodes = [np.concatenate(p) for p in perm_tiles]

    row = np.empty(n, dtype=np.int64)
    for c in range(C):
        row[perm_nodes[c]] = c * sz.SROWS + np.arange(sz.NPC)

    rows_of_srcs = row[srcs_sorted]          # source rows per CSR entry
    pair_of_srcs = rows_of_srcs >> 1
    par_of_srcs = (rows_of_srcs & 1).astype(np.int64)

    # even/odd source counts per node
    seg_id = np.repeat(np.arange(n), deg1)
    n_odd = np.bincount(seg_id, weights=par_of_srcs, minlength=n).astype(np.int64)
    n_even = deg1 - n_odd
    # sort each node's CSR segment: evens first, then odds
    order2 = np.lexsort((par_of_srcs, seg_id))
    pairs_s = pair_of_srcs[order2]

    # per-tile K (max over the 8 cores' tile-t nodes; program is SPMD)
    Ke, Ko = [], []
    for t in range(sz.NT):
        gnodes = np.concatenate(
            [perm_nodes[c][t * 128: t * 128 + sz.TSZ[t]] for c in range(C)])
        Ke.append(max(1, int(n_even[gnodes].max())))
        Ko.append(max(1, int(n_odd[gnodes].max())))

    # group tiles into gather instructions of <= GCAP slots
    groups = []
    cur, s = [], 0
    for t in range(sz.NT):
        kt = Ke[t] + Ko[t]
        if cur and s + kt > GCAP:
            groups.append(cur)
            cur, s = [], 0
        cur.append(t)
        s += kt
    if cur:
        groups.append(cur)

    # zero-pair block of core 0 (exists in every core's table copy)
    zpair0 = (sz.NPC) // 2
    zpairs = ZROWS // 2

    def tile_block(c, t):
        ke, ko = Ke[t], Ko[t]
        nodes = perm_nodes[c][t * 128: t * 128 + sz.TSZ[t]]
        spread = (np.arange(128)[:, None] * 7 + np.arange(ke + ko)[None, :]) % zpairs
        arr = (zpair0 + spread.astype(np.int64)).T.copy()   # [K,128] padding
        for j, n_ in enumerate(nodes):
            a = indptr[n_]
            ne = deg1[n_] - n_odd[n_]
            arr[0:ne, j] = pairs_s[a: a + ne]
            arr[ke: ke + n_odd[n_], j] = pairs_s[a + ne: a + deg1[n_]]
        return _wrap_idx(arr)

    idx_tensors = []
    for c in range(C):
        blocks = []
        for grp in groups:
            for t in grp:
                blocks.append(tile_block(c, t))
        idx_tensors.append(np.concatenate(blocks, axis=1))

    disp = np.zeros((C, 128, sz.NT), dtype=np.float32)
    for c in range(C):
        for t in range(sz.NT):
            disp[c, : sz.TSZ[t], t] = dis[perm_nodes[c][t * 128: t * 128 + sz.TSZ[t]]]

    return dict(perm_nodes=perm_nodes, Ke=Ke, Ko=Ko, groups=groups,
                idx=idx_tensors, disp=disp)


# -------------------------------------------------------------- device side

def build_program(nc, sz, meta):
    NPC, NT, TSZ = sz.NPC, sz.NT, sz.TSZ
    Ke, Ko, groups = meta["Ke"], meta["Ko"], meta["groups"]
    CW = sum((Ke[t] + Ko[t]) * 8 for t in range(NT))

    xT = nc.dram_tensor("xT", [IN, NPC], F32, kind="ExternalInput")
    idx = nc.dram_tensor("idx", [128, CW], I16, kind="ExternalInput")
    disp_d = nc.dram_tensor("disp", [128, NT], F32, kind="ExternalInput")
    dispw_d = nc.dram_tensor("dispw", [128, NT * H], F32, kind="ExternalInput")
    w1 = nc.dram_tensor("w1", [IN, H], F32, kind="ExternalInput")
    b1bc_d = nc.dram_tensor("b1bc", [128, H], F32, kind="ExternalInput")
    w2 = nc.dram_tensor("w2", [H, L], F32, kind="ExternalInput")
    b2_d = nc.dram_tensor("b2", [L, 1], F32, kind="ExternalInput")
    wd1 = nc.dram_tensor("wd1", [L, H], F32, kind="ExternalInput")
    bd1_d = nc.dram_tensor("bd1", [H, 1], F32, kind="ExternalInput")
    wd2 = nc.dram_tensor("wd2", [H, IN], F32, kind="ExternalInput")
    bd2_d = nc.dram_tensor("bd2", [128, 4], F32, kind="ExternalInput")
    ident_d = nc.dram_tensor("ident", [128, 128], F32, kind="ExternalInput")
    out = nc.dram_tensor("out", [NPC, IN], F32, kind="ExternalOutput")

    ACT = mybir.ActivationFunctionType
    ADD = mybir.AluOpType.add
    MULT = mybir.AluOpType.mult
    rg = [list(range(C))]

    table1 = nc.dram_tensor("table1", [sz.TROWS, H], F16, kind="Internal",
                            addr_space="Shared")
    table2 = nc.dram_tensor("table2", [sz.TROWS, H], F16, kind="Internal",
                            addr_space="Shared")

    with tile.TileContext(nc) as tc:
        with (
            tc.tile_pool(name="const", bufs=1) as cpool,
            tc.tile_pool(name="dram", bufs=1, space="DRAM") as dpool,
        ):
            cc1 = dpool.tile([sz.SROWS, H], F16)
            cc2 = dpool.tile([sz.SROWS, H], F16)
            ident = cpool.tile([128, 128], F32)
            nc.sync.dma_start(ident[:], ident_d.ap())
            ident16 = cpool.tile([128, 128], F16)
            nc.vector.tensor_copy(ident16[:], ident[:])
            b1bc = cpool.tile([128, H], F32)
            nc.sync.dma_start(b1bc[:], b1bc_d.ap())
            w1_sb = cpool.tile([128, 4, H], F32)
            nc.sync.dma_start(w1_sb[:], w1.ap().rearrange("(b p) f -> p b f", p=128))
            w2_sb = cpool.tile([H, L], F32)
            nc.sync.dma_start(w2_sb[:], w2.ap())
            b2_sb = cpool.tile([L, 1], F32)
            nc.sync.dma_start(b2_sb[:], b2_d.ap())
            wd1_sb = cpool.tile([L, H], F32)
            nc.sync.dma_start(wd1_sb[:], wd1.ap())
            bd1_sb = cpool.tile([H, 1], F32)
            nc.sync.dma_start(bd1_sb[:], bd1_d.ap())
            wd2_sb = cpool.tile([H, IN], F32)
            nc.sync.dma_start(wd2_sb[:], wd2.ap())
            bd2_sb = cpool.tile([128, 4], F32)
            nc.sync.dma_start(bd2_sb[:], bd2_d.ap())
            disp = cpool.tile([128, NT], F32)
            nc.sync.dma_start(disp[:], disp_d.ap())
            dispw = cpool.tile([128, NT, H], F32)
            nc.sync.dma_start(dispw[:], dispw_d.ap().rearrange(
                "p (t f) -> p t f", f=H))
            idx_sb = cpool.tile([128, CW], I16)
            nc.sync.dma_start(idx_sb[:], idx.ap())

            # zero rows of cc1/cc2 (padding gather targets)
            zb = ZROWS // 128
            zrow = cpool.tile([128, zb, H], F16)
            nc.vector.memset(zrow[:], 0.0)
            nc.sync.dma_start(
                cc1[NPC: sz.SROWS, :].rearrange("(p b) f -> p b f", p=128),
                zrow[:])
            nc.sync.dma_start(
                cc2[NPC: sz.SROWS, :].rearrange("(p b) f -> p b f", p=128),
                zrow[:])

            # ---------------- phase A: cc1 = fp16( dis * (x @ W1) )
            # x arrives pre-transposed; two big HWDGE loads (half the shard
            # each). Tiles are processed in batches of 8 per PSUM bank; one
            # wide DVE multiply applies the per-(tile,dst) dis scale and
            # casts to fp16, and one DMA stores the whole batch.
            NHALF = (NT + 1) // 2
            HROWS = NHALF * 128
            BT = 8
            with (
                tc.tile_pool(name="pa_x", bufs=2) as pax,
                tc.tile_pool(name="pa_sb", bufs=3) as pa,
                tc.tile_pool(name="pa_ps", bufs=3, space="PSUM") as pap,
            ):
                for half in range(2):
                    r0 = half * HROWS
                    r1 = min(NPC, (half + 1) * HROWS)
                    xh = pax.tile([128, 4, HROWS], F32, tag="xh")
                    nc.sync.dma_start(
                        xh[:, :, : r1 - r0],
                        xT.ap()[:, r0: r1].rearrange("(b k) d -> k b d", k=128))
                    tlo = half * NHALF
                    thi = min(NT, (half + 1) * NHALF)
                    for tb in range(tlo, thi, BT):
                        nb = min(BT, thi - tb)
                        ps = pap.tile([128, BT, H], F32, tag="ps")
                        for i in range(nb):
                            t = tb + i
                            pn = TSZ[t]
                            o = t * 128 - r0
                            for b in range(4):
                                nc.tensor.matmul(
                                    ps[:pn, i, :], xh[:, b, o: o + pn],
                                    w1_sb[:, b, :],
                                    start=(b == 0), stop=(b == 3),
                                )
                        ccb = pa.tile([128, BT, H], F16, tag="ccb")
                        nc.vector.tensor_tensor(
                            ccb[:, :nb, :], ps[:, :nb, :],
                            dispw[:, tb: tb + nb, :], op=MULT)
                        nfull = nb if tb + nb < NT else nb - 1
                        if nfull > 0:
                            nc.scalar.dma_start(
                                cc1[tb * 128: (tb + nfull) * 128, :].rearrange(
                                    "(t p) f -> p t f", p=128),
                                ccb[:, :nfull, :])
                        if tb + nb == NT:
                            pn = TSZ[NT - 1]
                            nc.scalar.dma_start(
                                cc1[(NT - 1) * 128: (NT - 1) * 128 + pn, :],
                                ccb[:pn, nb - 1, :])

            # ---------------- AllGather 1
            nc.gpsimd.collective_compute(
                "AllGather", mybir.AluOpType.bypass, replica_groups=rg,
                ins=[cc1.opt()], outs=[table1.ap()],
            )

            # ---------------- aggregation helper
            SMAX = max(sum(Ke[t] + Ko[t] for t in grp) for grp in groups)
            qctr = {"gi": 0}

            def agg_layer(table, epi, pg, pp, pu):
                in_ap = table.ap().rearrange("(a b) f -> a (b f)", b=2)
                col = 0
                for grp in groups:
                    S = sum(Ke[t] + Ko[t] for t in grp)
                    g = pg.tile([128, SMAX, 128], F16, tag="g")
                    nc.gpsimd.dma_gather(
                        out_ap=g[:, :S, :],
                        in_ap=in_ap,
                        idxs_ap=idx_sb[:, col: col + S * 8],
                        num_idxs=S * 128,
                        num_idxs_reg=S * 128,
                        elem_size=128,
                        single_packet=False,
                        queue_num=qctr["gi"] % NQ,
                    )
                    qctr["gi"] += 1
                    o = 0
                    for t in grp:
                        psum8 = pp.tile([128, 512], F32, tag="psum8")
                        first = True
                        for off, K, lo in ((o, Ke[t], 0), (o + Ke[t], Ko[t], 64)):
                            nmm = (K + 7) // 8
                            for j in range(nmm):
                                cnt = min(8, K - j * 8)
                                nc.tensor.matmul(
                                    psum8[:, : cnt * H],
                                    ident16[:],
                                    g[:, off + j * 8: off + j * 8 + cnt,
                                      lo: lo + H],
                                    start=first,
                                    stop=(lo == 64 and j == nmm - 1),
                                )
                                first = False
                        o += Ke[t] + Ko[t]
                        u = pu.tile([128, H], F32, tag="u")
                        nc.vector.tensor_reduce(
                            u[:],
                            psum8[:].rearrange("p (k f) -> p f k", k=8),
                            axis=mybir.AxisListType.X,
                            op=ADD,
                        )
                        epi(t, u)
                    col += S * 8

            # ---------------- layer 1 aggregation -> cc2
            with (
                tc.tile_pool(name="pc_g", bufs=4) as pcg,
                tc.tile_pool(name="pc_u", bufs=3) as pcu,
                tc.tile_pool(name="pc_ps", bufs=3, space="PSUM") as pcp,
            ):
                def epi_c(t, u):
                    # hs2 = dis * relu(dis*u + b1); relu commutes with the
                    # nonneg dis scale, so: t1 = dis*u (ACT), hpre = t1+b1
                    # (DVE add), hs2 = relu(dis*hpre... ) -- careful: the
                    # final dis must multiply relu(hpre), and ACT computes
                    # func(scale*in + bias), so Relu with scale=dis gives
                    # relu(dis*hpre) = dis*relu(hpre) since dis >= 0.
                    pn = TSZ[t]
                    t1 = pcu.tile([128, H], F32, tag="t1")
                    nc.scalar.activation(t1[:], u[:], ACT.Copy,
                                         scale=disp[:, t: t + 1])
                    hpre = pcu.tile([128, H], F32, tag="hpre")
                    nc.vector.tensor_tensor(hpre[:], t1[:], b1bc[:], op=ADD)
                    hs2 = pcu.tile([128, H], F16, tag="hs2")
                    nc.scalar.activation(hs2[:], hpre[:], ACT.Relu,
                                         scale=disp[:, t: t + 1])
                    nc.sync.dma_start(cc2[t * 128: t * 128 + pn, :],
                                      hs2[:pn, :])

                agg_layer(table1, epi_c, pcg, pcp, pcu)

            # ---------------- AllGather 2
            nc.gpsimd.collective_compute(
                "AllGather", mybir.AluOpType.bypass, replica_groups=rg,
                ins=[cc2.opt()], outs=[table2.ap()],
            )

            # ---------------- layer 2 aggregation + decoder -> out
            with (
                tc.tile_pool(name="pe_g", bufs=4) as peg,
                tc.tile_pool(name="pe_u", bufs=2) as peu,
                tc.tile_pool(name="pe_o", bufs=2) as peo,
                tc.tile_pool(name="pe_ps8", bufs=2, space="PSUM") as pep8,
                tc.tile_pool(name="pe_pss", bufs=3, space="PSUM") as peps,
                tc.tile_pool(name="pe_pso", bufs=3, space="PSUM") as pepo,
            ):
                def epi_e(t, u2):
                    pn = TSZ[t]
                    u2s = peu.tile([128, H], F32, tag="u2s")
                    nc.scalar.activation(u2s[:], u2[:], ACT.Copy,
                                         scale=disp[:, t: t + 1])
                    ps_uT = peps.tile([H, 128], F32, tag="ps_small")
                    nc.tensor.transpose(ps_uT[:], u2s[:], ident[:])
                    uT = peu.tile([H, 128], F32, tag="uT")
                    nc.vector.tensor_copy(uT[:], ps_uT[:])
                    ps_z = peps.tile([H, 128], F32, tag="ps_small")
                    nc.tensor.matmul(ps_z[:L, :], w2_sb[:], uT[:],
                                     start=True, stop=True)
                    zT = peu.tile([L, 128], F32, tag="zT")
                    nc.scalar.activation(zT[:], ps_z[:L, :], ACT.Relu,
                                         bias=b2_sb[:])
                    ps_d = peps.tile([H, 128], F32, tag="ps_small")
                    nc.tensor.matmul(ps_d[:], wd1_sb[:], zT[:],
                                     start=True, stop=True)
                    dT = peu.tile([H, 128], F32, tag="dT")
                    nc.scalar.activation(dT[:], ps_d[:], ACT.Relu,
                                         bias=bd1_sb[:])
                    osb = peo.tile([128, IN], F32, tag="osb")
                    for gblk in range(4):
                        ps_o = pepo.tile([128, 128], F32, tag="ps_oo")
                        nc.tensor.matmul(
                            ps_o[:],
                            wd2_sb[:, gblk * 128:(gblk + 1) * 128], dT[:],
                            start=True, stop=True,
                        )
                        oT = peu.tile([128, 128], F32, tag="oT")
                        nc.scalar.activation(
                            oT[:], ps_o[:], ACT.Sigmoid,
                            bias=bd2_sb[:, gblk: gblk + 1],
                        )
                        ps_on = pepo.tile([128, 128], F32, tag="ps_oo")
                        nc.tensor.transpose(ps_on[:], oT[:], ident[:])
                        nc.vector.tensor_copy(
                            osb[:, gblk * 128:(gblk + 1) * 128], ps_on[:]
                        )
                    nc.sync.dma_start(out.ap()[t * 128: t * 128 + pn, :],
                                      osb[:pn, :])

                agg_layer(table2, epi_e, peg, pep8, peu)
    return nc


def make_in_maps(sz, meta, x, W1, b1, W2, b2, Wd1, bd1, Wd2, bd2):
    b1bc = np.tile(b1[None, :], (128, 1)).astype(np.float32)
    bd2t = bd2.reshape(4, 128).T.copy().astype(np.float32)
    ident = np.eye(128, dtype=np.float32)
    in_maps = []
    for c in range(C):
        xp = np.ascontiguousarray(x[meta["perm_nodes"][c]].T)
        in_maps.append({
            "xT": xp,
            "idx": np.ascontiguousarray(meta["idx"][c]),
            "disp": np.ascontiguousarray(meta["disp"][c]),
            "dispw": np.ascontiguousarray(
                np.repeat(meta["disp"][c], H, axis=1).astype(np.float32)),
            "w1": W1, "b1bc": b1bc, "w2": W2,
            "b2": b2.reshape(L, 1).astype(np.float32),
            "wd1": Wd1, "bd1": bd1.reshape(H, 1).astype(np.float32),
            "wd2": Wd2, "bd2": bd2t, "ident": ident,
        })
    return in_maps


# ------------------------------------------------------------------- driver

def kernel(**inputs):
    x = np.ascontiguousarray(np.asarray(inputs["x"], dtype=np.float32))
    edge_index = np.asarray(inputs["edge_index"])
    args = [np.asarray(inputs[k], dtype=np.float32)
            for k in ["W1", "b1", "W2", "b2", "Wd1", "bd1", "Wd2", "bd2"]]

    sz = Sizes(x.shape[0])
    meta = _preprocess(sz, edge_index)

    nc = bacc.Bacc("TRN2", target_bir_lowering=False, debug=False,
                   num_devices=C, num_swdge_queues=NQ)
    build_program(nc, sz, meta)
    nc.compile()

    in_maps = make_in_maps(sz, meta, x, *args)

    trace = bool(int(os.environ.get("GCN_TRACE", "0")))
    res = run_bass_kernel_spmd(nc, in_maps, core_ids=list(range(C)), trace=trace)
    global LAST_RESULTS
    LAST_RESULTS = res

    out_full = np.empty((sz.N, IN), dtype=np.float32)
    for c in range(C):
        out_full[meta["perm_nodes"][c]] = res.results[c]["out"]
    return out_full


# revision 4
# speedup vs baseline: 1.2367x; 1.0925x over previous
"""GCN AutoEncoder on 8 Trainium2 NeuronCores (Bass/Tile) — v2.

Key insight from profiling: dma_gather is SWDGE descriptor-generation bound
(~8ns/index on one Q7 pair), and gathers on different queue_nums are
generated by different Q7 pairs IN PARALLEL (measured 2.8x with 4 queues).

Design:
  - Nodes degree-sorted and dealt round-robin into 8 per-core partitions
    (identical compile-time tile structure across cores). x is sent to each
    core pre-permuted AND pre-transposed, so phase A is 4 stationary-xT
    matmuls per tile with no on-chip transposes.
  - Tables are fp16, PAIR-packed: table row pair j = nodes (2j, 2j+1),
    256B per pair. Pair-index space (25512 < 32768) fits int16 with NO
    lo/hi bucket split. Both layers share ONE index tensor and ONE edge
    structure (identical gathers).
  - Per dst, source slots are split by source-row parity (even pair-half /
    odd pair-half). Aggregation sums the wanted 64-lane half of each
    gathered pair directly on the PE via identity-stationary matmuls into
    PSUM (moving slices [128, cnt, 0:64] / [64:128]), then one
    tensor_reduce. The dis_dst normalization factor is applied in the
    per-tile epilogue (per-partition scale); dis_src is prescaled into the
    table entries. Self-loops are ordinary slots.
  - Gathers round-robin over 4 SWDGE queues so 4 Q7 pairs generate
    descriptors concurrently.
  - AllGather moves fp16 tables (half the bytes of f32).
"""

import os

import numpy as np

import concourse.bass as bass
import concourse.bacc as bacc
import concourse.mybir as mybir
import concourse.tile as tile
from concourse.instruction_name_ordered_set import InstructionNameOrderedSet
from concourse.bass_utils import run_bass_kernel_spmd

F32 = mybir.dt.float32
F16 = mybir.dt.float16
I16 = mybir.dt.int16

IN = 512
H = 64
L = 32
C = 8          # cores
NQ = 4         # SWDGE queues
ZROWS = 128    # zero rows per core shard (64 zero pairs, padding targets)
GCAP = 72      # max slots per gather instruction (<= ~9216 descriptors)

LAST_RESULTS = None


class Sizes:
    def __init__(self, n):
        self.N = n
        self.NPC = n // C
        assert self.NPC * C == n and self.NPC % 2 == 0
        self.SROWS = self.NPC + ZROWS
        self.TROWS = self.SROWS * C
        self.PAIRS = self.TROWS // 2
        assert self.PAIRS < 32768
        self.NT = (self.NPC + 127) // 128
        self.TSZ = [128] * (self.NT - 1) + [self.NPC - (self.NT - 1) * 128]


def _wrap_idx(arr_k128):
    """Slot-major [K,128] -> wrapped [128, K*8] int16 (idx i at [i%16, i//16],
    replicated across the 8 groups of 16 partitions)."""
    flat = arr_k128.reshape(-1)
    w16 = flat.reshape(-1, 16).T
    return np.tile(w16, (8, 1)).astype(np.int16)


def _preprocess(sz, edge_index):
    n = sz.N
    src = np.asarray(edge_index[0], dtype=np.int64)
    dst = np.asarray(edge_index[1], dtype=np.int64)
    deg = np.bincount(dst, minlength=n).astype(np.int64) + 1
    dis = (1.0 / np.sqrt(deg.astype(np.float64))).astype(np.float32)

    # CSR over dst including self-loops
    srcs_all = np.concatenate([src, np.arange(n, dtype=np.int64)])
    dsts_all = np.concatenate([dst, np.arange(n, dtype=np.int64)])
    order = np.argsort(dsts_all, kind="stable")
    srcs_sorted = srcs_all[order]
    indptr = np.zeros(n + 1, dtype=np.int64)
    np.cumsum(np.bincount(dsts_all, minlength=n), out=indptr[1:])

    # partition nodes into tiles + assign pair-half parities.
    oorder = np.argsort(srcs_all, kind="stable")
    out_dst = dsts_all[oorder]
    out_ptr = np.zeros(n + 1, dtype=np.int64)
    np.cumsum(np.bincount(srcs_all, minlength=n), out=out_ptr[1:])
    odeg_all = np.diff(out_ptr)
    deg1 = np.diff(indptr)

    def deal(keys):
        """Sort nodes by key desc, deal per tile across cores."""
        pord = np.argsort(-keys, kind="stable")
        pt = [[] for _ in range(C)]
        off = 0
        for t in range(sz.NT):
            g = pord[off: off + sz.TSZ[t] * C]
            off += sz.TSZ[t] * C
            for c in range(C):
                pt[c].append(g[c * sz.TSZ[t]: (c + 1) * sz.TSZ[t]])
        return pt

    def balance(perm_tiles, parity_init=None, bal_init=None):
        """Greedy + swap-refined parity assignment under per-tile budgets."""
        tile_of = np.empty(n, dtype=np.int64)
        for c in range(C):
            for t in range(sz.NT):
                tile_of[perm_tiles[c][t]] = c * sz.NT + t
        ntiles = C * sz.NT
        budget_e = np.array([(len(perm_tiles[c][t]) + 1) // 2
                             for c in range(C) for t in range(sz.NT)])
        budget_o = np.array([len(perm_tiles[c][t])
                             for c in range(C) for t in range(sz.NT)]) - budget_e
        used_e = np.zeros(ntiles, dtype=np.int64)
        used_o = np.zeros(ntiles, dtype=np.int64)
        if parity_init is not None:
            # warm start: keep parities, repair per-tile budget violations
            parity = parity_init.copy()
            bal = bal_init.copy()
            for tl in range(ntiles):
                c, t = divmod(tl, sz.NT)
                nodes = perm_tiles[c][t]
                ev = nodes[parity[nodes] == 0]
                od = nodes[parity[nodes] == 1]
                while len(ev) > budget_e[tl]:
                    g = np.array([(1 - bal[out_dst[out_ptr[v]: out_ptr[v+1]]]).sum() for v in ev])
                    v = ev[np.argmin(g)]
                    parity[v] = 1
                    bal[out_dst[out_ptr[v]: out_ptr[v+1]]] -= 2
                    ev = ev[ev != v]
                while len(od) > budget_o[tl]:
                    g = np.array([(1 + bal[out_dst[out_ptr[v]: out_ptr[v+1]]]).sum() for v in od])
                    v = od[np.argmin(g)]
                    parity[v] = 0
                    bal[out_dst[out_ptr[v]: out_ptr[v+1]]] += 2
                    od = od[od != v]
            names_skip_pass1 = True
        else:
            names_skip_pass1 = False
            bal = np.zeros(n, dtype=np.int32)
            parity = np.zeros(n, dtype=np.int8)
        for node in (() if names_skip_pass1 else np.argsort(-odeg_all, kind="stable")):
            tl = tile_of[node]
            ds = out_dst[out_ptr[node]: out_ptr[node + 1]]
            b = bal[ds]
            score_e = int((b >= 0).sum()) - int((b < 0).sum())
            score_o = int((b <= 0).sum()) - int((b > 0).sum())
            pick_e = (score_e <= score_o)
            if pick_e and used_e[tl] >= budget_e[tl]:
                pick_e = False
            elif not pick_e and used_o[tl] >= budget_o[tl]:
                pick_e = True
            if pick_e:
                parity[node] = 0
                used_e[tl] += 1
                bal[ds] += 1
            else:
                parity[node] = 1
                used_o[tl] += 1
                bal[ds] -= 1
        for _ in range(6):
            nswap = 0
            for tl in range(ntiles):
                c, t = divmod(tl, sz.NT)
                nodes = perm_tiles[c][t]
                ev = nodes[parity[nodes] == 0]
                od = nodes[parity[nodes] == 1]
                ge = np.array([(1 - bal[out_dst[out_ptr[v]: out_ptr[v + 1]]]).sum()
                               for v in ev])
                go = np.array([(1 + bal[out_dst[out_ptr[v]: out_ptr[v + 1]]]).sum()
                               for v in od])
                eo = np.argsort(ge)
                oo = np.argsort(go)
                for i in range(min(len(ev), len(od))):
                    a_, b_ = ev[eo[i]], od[oo[i]]
                    da = out_dst[out_ptr[a_]: out_ptr[a_ + 1]]
                    db = out_dst[out_ptr[b_]: out_ptr[b_ + 1]]
                    gain = (1 - bal[da]).sum() + (1 + bal[db]).sum()
                    if gain >= 0:
                        break
                    parity[a_], parity[b_] = 1, 0
                    bal[da] -= 2
                    bal[db] += 2
                    nswap += 1
            if nswap == 0:
                break
        return parity, bal

    perm_tiles = deal(deg)
    parity, bal = balance(perm_tiles)
    # re-deal by the binding dimension max(n_even, n_odd), re-balance
    ne_d = (deg1 + bal) // 2
    no_d = (deg1 - bal) // 2
    perm_tiles = deal(np.maximum(ne_d, no_d) * 64 + np.minimum(ne_d, no_d))
    parity, bal = balance(perm_tiles, parity_init=parity, bal_init=bal)

    # order each tile: evens at positions 0,2,4..., odds at 1,3,5...
    for c in range(C):
        for t in range(sz.NT):
            nodes = perm_tiles[c][t]
            ev = nodes[parity[nodes] == 0]
            od = nodes[parity[nodes] == 1]
            arr = np.empty(len(nodes), dtype=np.int64)
            arr[0: 2 * len(ev): 2] = ev
            arr[1: 2 * len(od) + 1: 2] = od
            perm_tiles[c][t] = arr
    perm_nodes = [np.concatenate(p) for p in perm_tiles]

    row = np.empty(n, dtype=np.int64)
    for c in range(C):
        row[perm_nodes[c]] = c * sz.SROWS + np.arange(sz.NPC)

    rows_of_srcs = row[srcs_sorted]          # source rows per CSR entry
    pair_of_srcs = rows_of_srcs >> 1
    par_of_srcs = (rows_of_srcs & 1).astype(np.int64)

    # even/odd source counts per node
    seg_id = np.repeat(np.arange(n), deg1)
    n_odd = np.bincount(seg_id, weights=par_of_srcs, minlength=n).astype(np.int64)
    n_even = deg1 - n_odd
    # sort each node's CSR segment: evens first, then odds
    order2 = np.lexsort((par_of_srcs, seg_id))
    pairs_s = pair_of_srcs[order2]

    # per-tile K (max over the 8 cores' tile-t nodes; program is SPMD)
    Ke, Ko = [], []
    for t in range(sz.NT):
        gnodes = np.concatenate(
            [perm_nodes[c][t * 128: t * 128 + sz.TSZ[t]] for c in range(C)])
        Ke.append(max(1, int(n_even[gnodes].max())))
        Ko.append(max(1, int(n_odd[gnodes].max())))

    # group tiles into gather instructions of <= GCAP slots
    groups = []
    cur, s = [], 0
    for t in range(sz.NT):
        kt = Ke[t] + Ko[t]
        if cur and s + kt > GCAP:
            groups.append(cur)
            cur, s = [], 0
        cur.append(t)
        s += kt
    if cur:
        groups.append(cur)
    groups = groups[::-1]   # smallest groups first: warms the 4-queue
                            # pipeline with short gens before the big tiles

    # zero-pair block of core 0 (exists in every core's table copy)
    zpair0 = (sz.NPC) // 2
    zpairs = ZROWS // 2

    def tile_block(c, t):
        ke, ko = Ke[t], Ko[t]
        nodes = perm_nodes[c][t * 128: t * 128 + sz.TSZ[t]]
        spread = (np.arange(128)[:, None] * 7 + np.arange(ke + ko)[None, :]) % zpairs
        arr = (zpair0 + spread.astype(np.int64)).T.copy()   # [K,128] padding
        for j, n_ in enumerate(nodes):
            a = indptr[n_]
            ne = deg1[n_] - n_odd[n_]
            arr[0:ne, j] = pairs_s[a: a + ne]
            arr[ke: ke + n_odd[n_], j] = pairs_s[a + ne: a + deg1[n_]]
        return _wrap_idx(arr)

    idx_tensors = []
    for c in range(C):
        blocks = []
        for grp in groups:
            for t in grp:
                blocks.append(tile_block(c, t))
        idx_tensors.append(np.concatenate(blocks, axis=1))

    disp = np.zeros((C, 128, sz.NT), dtype=np.float32)
    for c in range(C):
        for t in range(sz.NT):
            disp[c, : sz.TSZ[t], t] = dis[perm_nodes[c][t * 128: t * 128 + sz.TSZ[t]]]

    return dict(perm_nodes=perm_nodes, Ke=Ke, Ko=Ko, groups=groups,
                idx=idx_tensors, disp=disp)


# -------------------------------------------------------------- device side

def build_program(nc, sz, meta):
    NPC, NT, TSZ = sz.NPC, sz.NT, sz.TSZ
    Ke, Ko, groups = meta["Ke"], meta["Ko"], meta["groups"]
    CW = sum((Ke[t] + Ko[t]) * 8 for t in range(NT))

    xT = nc.dram_tensor("xT", [IN, NPC], F32, kind="ExternalInput")
    idx = nc.dram_tensor("idx", [128, CW], I16, kind="ExternalInput")
    disp_d = nc.dram_tensor("disp", [128, NT], F32, kind="ExternalInput")
    dispw_d = nc.dram_tensor("dispw", [128, NT * H], F32, kind="ExternalInput")
    w1 = nc.dram_tensor("w1", [IN, H], F32, kind="ExternalInput")
    b1bc_d = nc.dram_tensor("b1bc", [128, H], F32, kind="ExternalInput")
    w2 = nc.dram_tensor("w2", [H, L], F32, kind="ExternalInput")
    b2_d = nc.dram_tensor("b2", [L, 1], F32, kind="ExternalInput")
    wd1 = nc.dram_tensor("wd1", [L, H], F32, kind="ExternalInput")
    bd1_d = nc.dram_tensor("bd1", [H, 1], F32, kind="ExternalInput")
    wd2 = nc.dram_tensor("wd2", [H, IN], F32, kind="ExternalInput")
    bd2_d = nc.dram_tensor("bd2", [128, 4], F32, kind="ExternalInput")
    ident_d = nc.dram_tensor("ident", [128, 128], F32, kind="ExternalInput")
    out = nc.dram_tensor("out", [NPC, IN], F32, kind="ExternalOutput")

    ACT = mybir.ActivationFunctionType
    ADD = mybir.AluOpType.add
    MULT = mybir.AluOpType.mult
    rg = [list(range(C))]

    table1 = nc.dram_tensor("table1", [sz.TROWS, H], F16, kind="Internal",
                            addr_space="Shared")
    table2 = nc.dram_tensor("table2", [sz.TROWS, H], F16, kind="Internal",
                            addr_space="Shared")

    with tile.TileContext(nc) as tc:
        with (
            tc.tile_pool(name="const", bufs=1) as cpool,
            tc.tile_pool(name="dram", bufs=1, space="DRAM") as dpool,
        ):
            cc1 = dpool.tile([sz.SROWS, H], F16)
            cc2 = dpool.tile([sz.SROWS, H], F16)
            ident = cpool.tile([128, 128], F32)
            nc.sync.dma_start(ident[:], ident_d.ap())
            ident16 = cpool.tile([128, 128], F16)
            nc.vector.tensor_copy(ident16[:], ident[:])
            b1bc = cpool.tile([128, H], F32)
            nc.sync.dma_start(b1bc[:], b1bc_d.ap())
            w1_sb = cpool.tile([128, 4, H], F32)
            nc.sync.dma_start(w1_sb[:], w1.ap().rearrange("(b p) f -> p b f", p=128))
            w2_sb = cpool.tile([H, L], F32)
            nc.sync.dma_start(w2_sb[:], w2.ap())
            b2_sb = cpool.tile([L, 1], F32)
            nc.sync.dma_start(b2_sb[:], b2_d.ap())
            wd1_sb = cpool.tile([L, H], F32)
            nc.sync.dma_start(wd1_sb[:], wd1.ap())
            bd1_sb = cpool.tile([H, 1], F32)
            nc.sync.dma_start(bd1_sb[:], bd1_d.ap())
            wd2_sb = cpool.tile([H, IN], F32)
            nc.sync.dma_start(wd2_sb[:], wd2.ap())
            bd2_sb = cpool.tile([128, 4], F32)
            nc.sync.dma_start(bd2_sb[:], bd2_d.ap())
            disp = cpool.tile([128, NT], F32)
            nc.sync.dma_start(disp[:], disp_d.ap())
            dispw = cpool.tile([128, NT, H], F32)
            nc.scalar.dma_start(dispw[:], dispw_d.ap().rearrange(
                "p (t f) -> p t f", f=H))
            idx_sb = cpool.tile([128, CW], I16)
            nc.scalar.dma_start(idx_sb[:], idx.ap())

            # zero rows of cc1/cc2 (padding gather targets)
            zb = ZROWS // 128
            zrow = cpool.tile([128, zb, H], F16)
            nc.vector.memset(zrow[:], 0.0)
            nc.sync.dma_start(
                cc1[NPC: sz.SROWS, :].rearrange("(p b) f -> p b f", p=128),
                zrow[:])
            nc.sync.dma_start(
                cc2[NPC: sz.SROWS, :].rearrange("(p b) f -> p b f", p=128),
                zrow[:])

            # ---------------- phase A: cc1 = fp16( dis * (x @ W1) )
            # x arrives pre-transposed; two big HWDGE loads (half the shard
            # each). Tiles are processed in batches of 8 per PSUM bank; one
            # wide DVE multiply applies the per-(tile,dst) dis scale and
            # casts to fp16, and one DMA stores the whole batch.
            NHALF = (NT + 1) // 2
            HROWS = NHALF * 128
            BT = 8
            with (
                tc.tile_pool(name="pa_x", bufs=2) as pax,
                tc.tile_pool(name="pa_sb", bufs=3) as pa,
                tc.tile_pool(name="pa_ps", bufs=3, space="PSUM") as pap,
            ):
                for half in range(2):
                    r0 = half * HROWS
                    r1 = min(NPC, (half + 1) * HROWS)
                    xh = pax.tile([128, 4, HROWS], F32, tag="xh")
                    nc.sync.dma_start(
                        xh[:, :, : r1 - r0],
                        xT.ap()[:, r0: r1].rearrange("(b k) d -> k b d", k=128))
                    tlo = half * NHALF
                    thi = min(NT, (half + 1) * NHALF)
                    for tb in range(tlo, thi, BT):
                        nb = min(BT, thi - tb)
                        ps = pap.tile([128, BT, H], F32, tag="ps")
                        for i in range(nb):
                            t = tb + i
                            pn = TSZ[t]
                            o = t * 128 - r0
                            for b in range(4):
                                nc.tensor.matmul(
                                    ps[:pn, i, :], xh[:, b, o: o + pn],
                                    w1_sb[:, b, :],
                                    start=(b == 0), stop=(b == 3),
                                )
                        ccb = pa.tile([128, BT, H], F16, tag="ccb")
                        nc.vector.tensor_tensor(
                            ccb[:, :nb, :], ps[:, :nb, :],
                            dispw[:, tb: tb + nb, :], op=MULT)
                        nfull = nb if tb + nb < NT else nb - 1
                        if nfull > 0:
                            nc.sync.dma_start(
                                cc1[tb * 128: (tb + nfull) * 128, :].rearrange(
                                    "(t p) f -> p t f", p=128),
                                ccb[:, :nfull, :])
                        if tb + nb == NT:
                            pn = TSZ[NT - 1]
                            nc.sync.dma_start(
                                cc1[(NT - 1) * 128: (NT - 1) * 128 + pn, :],
                                ccb[:pn, nb - 1, :])

            # ---------------- AllGather 1
            nc.gpsimd.collective_compute(
                "AllGather", mybir.AluOpType.bypass, replica_groups=rg,
                ins=[cc1.opt()], outs=[table1.ap()],
            )

            # ---------------- aggregation helper
            SMAX = max(sum(Ke[t] + Ko[t] for t in grp) for grp in groups)
            qctr = {"gi": 0}

            def agg_layer(table, epi, pg, pp, pu):
                in_ap = table.ap().rearrange("(a b) f -> a (b f)", b=2)
                col = 0
                for grp in groups:
                    S = sum(Ke[t] + Ko[t] for t in grp)
                    g = pg.tile([128, SMAX, 128], F16, tag="g")
                    nc.gpsimd.dma_gather(
                        out_ap=g[:, :S, :],
                        in_ap=in_ap,
                        idxs_ap=idx_sb[:, col: col + S * 8],
                        num_idxs=S * 128,
                        num_idxs_reg=S * 128,
                        elem_size=128,
                        single_packet=False,
                        queue_num=qctr["gi"] % NQ,
                    )
                    qctr["gi"] += 1
                    o = 0
                    for t in grp:
                        psum8 = pp.tile([128, 512], F32, tag="psum8")
                        first = True
                        for off, K, lo in ((o, Ke[t], 0), (o + Ke[t], Ko[t], 64)):
                            nmm = (K + 7) // 8
                            for j in range(nmm):
                                cnt = min(8, K - j * 8)
                                nc.tensor.matmul(
                                    psum8[:, : cnt * H],
                                    ident16[:],
                                    g[:, off + j * 8: off + j * 8 + cnt,
                                      lo: lo + H],
                                    start=first,
                                    stop=(lo == 64 and j == nmm - 1),
                                )
                                first = False
                        o += Ke[t] + Ko[t]
                        u = pu.tile([128, H], F32, tag="u")
                        nc.vector.tensor_reduce(
                            u[:],
                            psum8[:].rearrange("p (k f) -> p f k", k=8),
                            axis=mybir.AxisListType.X,
                            op=ADD,
                        )
                        epi(t, u)
                    col += S * 8

            # ---------------- layer 1 aggregation -> cc2
            with (
                tc.tile_pool(name="pc_g", bufs=4) as pcg,
                tc.tile_pool(name="pc_u", bufs=3) as pcu,
                tc.tile_pool(name="pc_ps", bufs=3, space="PSUM") as pcp,
            ):
                def epi_c(t, u):
                    # hs2 = dis * relu(dis*u + b1); relu commutes with the
                    # nonneg dis scale, so: t1 = dis*u (ACT), hpre = t1+b1
                    # (DVE add), hs2 = relu(dis*hpre... ) -- careful: the
                    # final dis must multiply relu(hpre), and ACT computes
                    # func(scale*in + bias), so Relu with scale=dis gives
                    # relu(dis*hpre) = dis*relu(hpre) since dis >= 0.
                    pn = TSZ[t]
                    t1 = pcu.tile([128, H], F32, tag="t1")
                    nc.scalar.activation(t1[:], u[:], ACT.Copy,
                                         scale=disp[:, t: t + 1])
                    hpre = pcu.tile([128, H], F32, tag="hpre")
                    nc.vector.tensor_tensor(hpre[:], t1[:], b1bc[:], op=ADD)
                    hs2 = pcu.tile([128, H], F16, tag="hs2")
                    nc.scalar.activation(hs2[:], hpre[:], ACT.Relu,
                                         scale=disp[:, t: t + 1])
                    nc.sync.dma_start(cc2[t * 128: t * 128 + pn, :],
                                      hs2[:pn, :])

                agg_layer(table1, epi_c, pcg, pcp, pcu)

            # ---------------- AllGather 2
            nc.gpsimd.collective_compute(
                "AllGather", mybir.AluOpType.bypass, replica_groups=rg,
                ins=[cc2.opt()], outs=[table2.ap()],
            )

            # ---------------- layer 2 aggregation + decoder -> out
            with (
                tc.tile_pool(name="pe_g", bufs=4) as peg,
                tc.tile_pool(name="pe_u", bufs=2) as peu,
                tc.tile_pool(name="pe_o", bufs=2) as peo,
                tc.tile_pool(name="pe_ps8", bufs=2, space="PSUM") as pep8,
                tc.tile_pool(name="pe_pss", bufs=3, space="PSUM") as peps,
                tc.tile_pool(name="pe_pso", bufs=3, space="PSUM") as pepo,
            ):
                def epi_e(t, u2):
                    pn = TSZ[t]
                    u2s = peu.tile([128, H], F32, tag="u2s")
                    nc.scalar.activation(u2s[:], u2[:], ACT.Copy,
                                         scale=disp[:, t: t + 1])
                    ps_uT = peps.tile([H, 128], F32, tag="ps_small")
                    nc.tensor.transpose(ps_uT[:], u2s[:], ident[:])
                    uT = peu.tile([H, 128], F32, tag="uT")
                    nc.vector.tensor_copy(uT[:], ps_uT[:])
                    ps_z = peps.tile([H, 128], F32, tag="ps_small")
                    nc.tensor.matmul(ps_z[:L, :], w2_sb[:], uT[:],
                                     start=True, stop=True)
                    zT = peu.tile([L, 128], F32, tag="zT")
                    nc.scalar.activation(zT[:], ps_z[:L, :], ACT.Relu,
                                         bias=b2_sb[:])
                    ps_d = peps.tile([H, 128], F32, tag="ps_small")
                    nc.tensor.matmul(ps_d[:], wd1_sb[:], zT[:],
                                     start=True, stop=True)
                    dT = peu.tile([H, 128], F32, tag="dT")
                    nc.scalar.activation(dT[:], ps_d[:], ACT.Relu,
                                         bias=bd1_sb[:])
                    osb = peo.tile([128, IN], F32, tag="osb")
                    for gblk in range(4):
                        ps_o = pepo.tile([128, 128], F32, tag="ps_oo")
                        nc.tensor.matmul(
                            ps_o[:],
                            wd2_sb[:, gblk * 128:(gblk + 1) * 128], dT[:],
                            start=True, stop=True,
                        )
                        oT = peu.tile([128, 128], F32, tag="oT")
                        nc.scalar.activation(
                            oT[:], ps_o[:], ACT.Sigmoid,
                            bias=bd2_sb[:, gblk: gblk + 1],
                        )
                        ps_on = pepo.tile([128, 128], F32, tag="ps_oo")
                        nc.tensor.transpose(ps_on[:], oT[:], ident[:])
                        nc.vector.tensor_copy(
                            osb[:, gblk * 128:(gblk + 1) * 128], ps_on[:]
                        )
                    nc.sync.dma_start(out.ap()[t * 128: t * 128 + pn, :],
                                      osb[:pn, :])

                agg_layer(table2, epi_e, peg, pep8, peu)
    return nc


def make_in_maps(sz, meta, x, W1, b1, W2, b2, Wd1, bd1, Wd2, bd2):
    b1bc = np.tile(b1[None, :], (128, 1)).astype(np.float32)
    bd2t = bd2.reshape(4, 128).T.copy().astype(np.float32)
    ident = np.eye(128, dtype=np.float32)
    in_maps = []
    for c in range(C):
        xp = np.ascontiguousarray(x[meta["perm_nodes"][c]].T)
        in_maps.append({
            "xT": xp,
            "idx": np.ascontiguousarray(meta["idx"][c]),
            "disp": np.ascontiguousarray(meta["disp"][c]),
            "dispw": np.ascontiguousarray(
                np.repeat(meta["disp"][c], H, axis=1).astype(np.float32)),
            "w1": W1, "b1bc": b1bc, "w2": W2,
            "b2": b2.reshape(L, 1).astype(np.float32),
            "wd1": Wd1, "bd1": bd1.reshape(H, 1).astype(np.float32),
            "wd2": Wd2, "bd2": bd2t, "ident": ident,
        })
    return in_maps


# ------------------------------------------------------------------- driver

def kernel(**inputs):
    x = np.ascontiguousarray(np.asarray(inputs["x"], dtype=np.float32))
    edge_index = np.asarray(inputs["edge_index"])
    args = [np.asarray(inputs[k], dtype=np.float32)
            for k in ["W1", "b1", "W2", "b2", "Wd1", "bd1", "Wd2", "bd2"]]

    sz = Sizes(x.shape[0])
    meta = _preprocess(sz, edge_index)

    nc = bacc.Bacc("TRN2", target_bir_lowering=False, debug=False,
                   num_devices=C, num_swdge_queues=NQ)
    build_program(nc, sz, meta)
    nc.compile()

    in_maps = make_in_maps(sz, meta, x, *args)

    trace = bool(int(os.environ.get("GCN_TRACE", "0")))
    res = run_bass_kernel_spmd(nc, in_maps, core_ids=list(range(C)), trace=trace)
    global LAST_RESULTS
    LAST_RESULTS = res

    out_full = np.empty((sz.N, IN), dtype=np.float32)
    for c in range(C):
        out_full[meta["perm_nodes"][c]] = res.results[c]["out"]
    return out_full
